# revision 35
# baseline (speedup 1.0000x reference)
"""Trainium2 Bass kernel for the CapsuleLayer routing problem.

Strategy (8 NeuronCores, shard the input-capsule dim I):
  - Each core owns I_loc = 256 input capsules; votes[b, i_loc, d, a] are computed
    on the TensorEngine with block-diagonal x as the stationary operand and the
    capsule weights streamed from HBM once (8 MB/core), then kept in SBUF as
    bf16 in layout [partition=(j, b16), free=(g, a, d)] (i = 8*g + j).
  - Routing iterations run fully on-chip: softmax on DVE/ACT, the
    route-weighted i-reduction via PSUM-accumulated matmuls against a
    0/1 selection matrix, and the agreement update (sum over atoms a) as a
    bf16 add-tree on DVE.
  - Per-iteration cross-core reduction of the tiny preact partials
    ([32, 512] = 64KB) uses an AllReduce collective; the final iteration's
    partials go straight to HBM and the host does the last bias+squash in
    fp64.
"""

import functools

import numpy as np
import ml_dtypes

import concourse.bass as bass
import concourse.tile as tile
from concourse import bacc, mybir
from concourse import bass_utils

N_CORES = 8
B, I, C, D, A = 32, 2048, 16, 32, 16
I_LOC = I // N_CORES          # 256 capsules per core
G = I_LOC // 8                # 32 groups of 8 capsules
O = A * D                     # 512, free layout is (a, d) with d innermost

F32 = mybir.dt.float32
BF16 = mybir.dt.bfloat16
_nbf16 = ml_dtypes.bfloat16


def _build(num_routing: int, dbg: bool = False, reps: int = 1,
           opts: frozenset = frozenset()):
    nc = bacc.Bacc("TRN2", target_bir_lowering=False, debug=False,
                   enable_asserts=True, num_devices=N_CORES)

    in_dt = BF16 if "w_bf16" in opts else F32
    if "wb2" in opts:
        # partition-major weight layout: wh[p, g, o] = w[g*128 + p, o], so a
        # multi-group DMA reads contiguous >=2KB per partition line
        w_in = nc.dram_tensor("wh", [128, G, O], in_dt, kind="ExternalInput").ap()
    else:
        w_in = nc.dram_tensor("w", [I_LOC * C, O], in_dt, kind="ExternalInput").ap()
    XB = 8 if "xd8" in opts else 4
    if "xdc" in opts:
        # pre-transposed + group-batched for contiguous >=2KB DMA lines
        xd_in = nc.dram_tensor("xdc", [G // XB, 128, XB, 2, 128], in_dt,
                               kind="ExternalInput").ap()
    else:
        xd_in = nc.dram_tensor("xd", [G, 2, 128, 128], in_dt,
                               kind="ExternalInput").ap()
    if "d32" in opts:
        xt32_in = nc.dram_tensor("xt32", [128, G, 32], BF16,
                                 kind="ExternalInput").ap()
    s_in = nc.dram_tensor("s", [128, 2, B], BF16, kind="ExternalInput").ap()
    s32_in = nc.dram_tensor("s32", [128, 2, B], BF16, kind="ExternalInput").ap()
    sh_in = nc.dram_tensor("sh", [128, 16], BF16, kind="ExternalInput").ap()
    shx_in = nc.dram_tensor("shx", [128, 2, 32], BF16, kind="ExternalInput").ap()
    sh32_in = nc.dram_tensor("sh32", [128, 16], BF16, kind="ExternalInput").ap()
    repl_in = nc.dram_tensor("repl", [16, 128], BF16, kind="ExternalInput").ap()
    xt_in = nc.dram_tensor("xt", [G, 128, 2, 16], BF16, kind="ExternalInput").ap()
    bias2_in = nc.dram_tensor("bias2", [1, O], BF16, kind="ExternalInput").ap()
    ones1_in = nc.dram_tensor("ones1", [1, 128], BF16, kind="ExternalInput").ap()
    bias_in = nc.dram_tensor("biasb", [128, O], F32, kind="ExternalInput").ap()
    outp = nc.dram_tensor("outp", [B, O], F32, kind="ExternalOutput").ap()
    if dbg:
        dbg_prep = nc.dram_tensor("dbg_prep", [2, 128, O], F32, kind="ExternalOutput").ap()
        dbg_actb = nc.dram_tensor("dbg_actb", [2, 128, O], BF16, kind="ExternalOutput").ap()
        dbg_L = nc.dram_tensor("dbg_L", [2, 128, G * D], F32, kind="ExternalOutput").ap()
        dbg_R = nc.dram_tensor("dbg_R", [2, 128, G * D], BF16, kind="ExternalOutput").ap()
        dbg_ar = nc.dram_tensor("dbg_ar", [B, O], F32, kind="ExternalOutput").ap()

    Exp = mybir.ActivationFunctionType.Exp
    Square = mybir.ActivationFunctionType.Square
    Sqrt = mybir.ActivationFunctionType.Sqrt
    add = mybir.AluOpType.add
    sub = mybir.AluOpType.subtract
    mult = mybir.AluOpType.mult
    amax = mybir.AluOpType.max
    AX = mybir.AxisListType.X

    with tile.TileContext(nc) as tc:
        with (
            tc.tile_pool(name="persist", bufs=1) as persist,
            tc.tile_pool(name="wpool",
                         bufs=(4 if "hsplit" in opts else 2)
                         if "wb2" in opts else 3) as wpool,
            tc.tile_pool(name="xpool",
                         bufs=(3 if "hsplit" in opts else 2)
                         if "xdc" in opts else 3) as xpool,
            tc.tile_pool(name="pspool",
                         bufs=(4 if "ccs1" in opts else 5)
                         if "hsplit" in opts
                         else (3 if "d32" in opts else 4),
                         space="PSUM") as pspool,
            tc.tile_pool(name="papool", bufs=1, space="PSUM") as papool,
            tc.tile_pool(name="stage", bufs=2) as stage,
            tc.tile_pool(name="rpool", bufs=2) as rpool,
            tc.tile_pool(name="wvpool",
                         bufs=2 if any(o.startswith("pswv") for o in opts)
                         else 4) as wvpool,
            tc.tile_pool(name="wvppool", bufs=1) as wvppool,
            tc.tile_pool(name="upool", bufs=1) as upool,
            tc.tile_pool(name="uppool", bufs=1) as uppool,
            tc.tile_pool(name="small", bufs=3) as small,
            tc.tile_pool(name="pre32p", bufs=1) as pre32p,
            tc.tile_pool(name="dram", bufs=2, space="DRAM") as dram,
        ):
            V = [persist.tile([128, G, A, D], BF16, tag=f"V{h}", name=f"V{h}")
                 for h in range(2)]
            L = [persist.tile([128, G, D], F32, tag=f"L{h}", name=f"L{h}")
                 for h in range(2)]
            s_sb = persist.tile([128, 2, B], BF16, tag="s", name="s_sb")
            s32_sb = persist.tile([128, 2, B], BF16, tag="s32", name="s32_sb")
            sh_sb = persist.tile([128, 16], BF16, tag="sh", name="sh_sb")
            shx_sb = persist.tile([128, 2, 32], BF16, tag="shx", name="shx_sb")
            nc.sync.dma_start(shx_sb[:], shx_in[:])
            sh32_sb = persist.tile([128, 16], BF16, tag="sh32", name="sh32_sb")
            repl_sb = persist.tile([16, 128], BF16, tag="repl", name="repl_sb")
            nc.sync.dma_start(repl_sb[:], repl_in[:])
            if "d32" in opts:
                xt32_sb = persist.tile([128, G, 32], BF16, tag="xt32",
                                       name="xt32_sb")
                nc.sync.dma_start(xt32_sb[:], xt32_in[:])
            else:
                xt_sb = persist.tile([128, G, 2, 16], BF16, tag="xt",
                                     name="xt_sb")
                nc.sync.dma_start(xt_sb[:], xt_in.rearrange("g p h b -> p g h b"))
            bias2_sb = persist.tile([1, O], BF16, tag="bias2", name="bias2_sb")
            nc.sync.dma_start(bias2_sb[:], bias2_in[:])
            ones1_sb = persist.tile([1, 128], BF16, tag="ones1", name="ones1_sb")
            nc.sync.dma_start(ones1_sb[:], ones1_in[:])
            bias_sb = persist.tile([128, A, D], F32, tag="bias", name="bias_sb")

            nc.sync.dma_start(s_sb[:], s_in[:])
            nc.sync.dma_start(s32_sb[:], s32_in[:])
            nc.sync.dma_start(sh_sb[:], sh_in[:])
            nc.sync.dma_start(sh32_sb[:], sh32_in[:])
            nc.sync.dma_start(bias_sb[:], bias_in.rearrange("p (a d) -> p a d", a=A))
            for _rep in range(reps):
              for h in range(2):
                nc.vector.memset(L[h][:], 0.0)

              # ---- votes production ----
              dense0 = "dense0" in opts
              d32 = "d32" in opts
              if dense0:
                  if d32:
                      paD32 = papool.tile([B, O], F32, tag="pa32", name="paD32")
                      paD = [paD32[bass.ts(h, 16)] for h in range(2)]
                  else:
                      paD = [papool.tile([16, O], F32, tag=f"pa{h}", name="paD")[:]
                             for h in range(2)]
              def votes_loop(hs, copy_eng=None):
                  xdq = None
                  wt2 = None
                  for g in range(G):
                      if "wb2" in opts:
                          WB = 4 if "wb4" in opts else 2
                          if g % WB == 0:
                              wt2 = wpool.tile([128, WB, O], in_dt, tag="wt",
                                               name="wt")
                              nc.sync.dma_start(wt2[:],
                                                w_in[:, bass.ts(g // WB, WB)])
                          wt = wt2[:, g % WB]
                      else:
                          wt = wpool.tile([128, O], in_dt, tag="wt", name="wt")[:]
                          nc.sync.dma_start(wt[:], w_in[bass.ts(g, 128), :])
                      if "xdc" in opts:
                          if g % XB == 0:
                              xdq = xpool.tile([128, XB, 2, 128], in_dt,
                                               tag="xdt", name="xdt")
                              nc.sync.dma_start(xdq[:], xd_in[g // XB])
                          xdt = xdq[:, g % XB]
                      else:
                          xdt = xpool.tile([128, 2, 128], in_dt, tag="xdt",
                                           name="xdt")[:]
                          nc.sync.dma_start(xdt[:],
                                            xd_in[g].rearrange("t p m -> p t m"))
                      if dense0 and 0 in hs:
                          # iter-0 preact accumulates here: route is uniform
                          # 1/D, folded into the pre-scaled xT (exact in bf16)
                          if d32:
                              nc.tensor.matmul(paD32[:], lhsT=xt32_sb[:, g, :],
                                               rhs=wt[:], start=(g == 0),
                                               stop=(g == G - 1))
                          else:
                              for h in range(2):
                                  nc.tensor.matmul(paD[h][:],
                                                   lhsT=xt_sb[:, g, h, :],
                                                   rhs=wt[:], start=(g == 0),
                                                   stop=(g == G - 1))
                      for h in hs:
                          ps = pspool.tile([128, O], F32, tag="ps", name="ps")
                          nc.tensor.matmul(ps[:], lhsT=xdt[:, h, :], rhs=wt[:],
                                           start=True, stop=True)
                          dst = V[h][:, g]
                          src = ps[:].rearrange("p (a d) -> p a d", a=A)
                          if copy_eng == "act":
                              nc.scalar.copy(dst[:], src)
                          elif copy_eng == "pool":
                              nc.gpsimd.tensor_copy(dst[:], src)
                          else:
                              if "hsplit" in opts:
                                  dve_copy = g % 2 == 1
                              else:
                                  dve_copy = ((g % 4 == 3) if "tail1" in opts
                                              else (g % 2 == 1))
                              if "act_copies" in opts or not dve_copy:
                                  nc.scalar.copy(dst[:], src)
                              else:
                                  nc.vector.tensor_copy(dst[:], src)

              hsplit = "hsplit" in opts
              if hsplit:
                  assert dense0 and d32 and "tail1" in opts
                  votes_loop([0])
              else:
                  votes_loop([0, 1])

              if hsplit and "votes_only" not in opts:
                  # Restructured routing: one merged [B, O] collective per
                  # iteration; it0's collective + squash/update overlap the
                  # h=1 votes loop (PE/ACT/DMA there, DVE/Pool here).
                  cc_dt = BF16 if "cc_bf16" in opts else F32
                  uG0 = G
                  wvG0 = G
                  WVC = 4
                  for o in opts:
                      if o.startswith("psu"):
                          uG0 = int(o[3:])
                      elif o.startswith("pswv"):
                          wvG0 = int(o[4:])

                  def cc_reduce32(pre_psum):
                      """[B,O] PSUM partial -> all-reduced DRAM buffer."""
                      pre32 = pre32p.tile([B, O], cc_dt, tag="pre32",
                                          name="pre32")
                      nc.scalar.copy(pre32[:], pre_psum[:])
                      inb = dram.tile([B, O], cc_dt, tag="arin32", name="arin32")
                      outb = dram.tile([B, O], cc_dt, tag="arout32",
                                       name="arout32", addr_space="Shared")
                      nc.sync.dma_start(inb[:], pre32[:])
                      if "nocc" in opts:
                          nc.sync.dma_start(outb[:], inb[:])
                      else:
                          nc.gpsimd.collective_compute(
                              "AllReduce", add,
                              replica_groups=[list(range(N_CORES))],
                              ins=[inb[:].opt()], outs=[outb[:].opt()])
                      return outb

                  outb32_0 = cc_reduce32(paD32)
                  outb_cur = [outb32_0[bass.ts(h, 16)] for h in range(2)]
                  votes_loop([1])

                  for it in range(num_routing):
                      # ---- phase B of `it`: squash + agreement update ----
                      if it < num_routing - 1:
                          for h in range(2):
                              ob_sb = small.tile([16, O], cc_dt, tag=f"ob{h}",
                                                 name="ob_sb")
                              nc.sync.dma_start(ob_sb[:], outb_cur[h])
                              prep_ps = papool.tile([128, O], F32,
                                                    tag="prps",
                                                    name="prep_ps")
                              nc.tensor.matmul(prep_ps[:], lhsT=repl_sb[:],
                                               rhs=ob_sb[:], start=True,
                                               stop=False)
                              nc.tensor.matmul(prep_ps[:], lhsT=ones1_sb[:],
                                               rhs=bias2_sb[:], start=False,
                                               stop=True)
                              t2 = prep_ps[:].rearrange("p (a d) -> p a d", a=A)
                              sq = stage.tile([128, A, D], F32, tag="sq",
                                              name="sq")
                              nc.scalar.activation(sq[:], t2, Square)
                              n2 = small.tile([128, D], F32, tag="n2", name="n2")
                              nc.vector.tensor_reduce(
                                  n2[:], sq[:].rearrange("p a d -> p d a"),
                                  axis=AX, op=add)
                              nrm = small.tile([128, D], F32, tag="nrm",
                                               name="nrm")
                              nc.scalar.activation(nrm[:], n2[:], Sqrt)
                              den = small.tile([128, D], F32, tag="den",
                                               name="den")
                              nc.vector.tensor_scalar_add(den[:], n2[:], 1.0)
                              rc2 = small.tile([128, D], F32, tag="rc2",
                                               name="rc2")
                              nc.vector.reciprocal(rc2[:], den[:])
                              fac = small.tile([128, D], F32, tag="fac",
                                               name="fac")
                              nc.vector.tensor_tensor(fac[:], nrm[:], rc2[:],
                                                      mult)
                              actb = stage.tile([128, A, D], BF16, tag="actb",
                                                name="actb")
                              nc.vector.tensor_tensor(
                                  actb[:], t2,
                                  fac[:, None, :].to_broadcast([128, A, D]),
                                  mult)
                              if uG0 < G:
                                  up = uppool.tile([128, G - uG0, A, D], BF16,
                                                   tag="up", name="up")
                                  nc.gpsimd.tensor_tensor(
                                      up[:], V[h][:, uG0:],
                                      actb[:, None, :, :]
                                      .to_broadcast([128, G - uG0, A, D]), mult)
                              u = upool.tile([128, uG0, A, D], BF16,
                                             tag="u", name="u")
                              nc.vector.tensor_tensor(
                                  u[:], V[h][:, :uG0],
                                  actb[:, None, :, :]
                                  .to_broadcast([128, uG0, A, D]), mult)
                              half = A // 2
                              while half >= 1:
                                  if uG0 < G:
                                      nc.gpsimd.tensor_tensor(
                                          up[:, :, 0:half], up[:, :, 0:half],
                                          up[:, :, half:2 * half], add)
                                  nc.vector.tensor_tensor(
                                      u[:, :, 0:half], u[:, :, 0:half],
                                      u[:, :, half:2 * half], add)
                                  half //= 2
                              nc.vector.tensor_tensor(L[h][:, :uG0],
                                                      L[h][:, :uG0],
                                                      u[:, :, 0, :], add)
                              if uG0 < G:
                                  nc.gpsimd.tensor_tensor(L[h][:, uG0:],
                                                          L[h][:, uG0:],
                                                          up[:, :, 0, :], add)
                      # ---- phase A of `it+1`: softmax + wv + reduce ----
                      if it == num_routing - 1:
                          break
                      nxt_last = it + 1 == num_routing - 1
                      ccs1 = "ccs1" in opts and not nxt_last
                      if not ccs1:
                          pa32 = papool.tile([B, O], F32, tag="pa32",
                                             name="pa32")
                      nxt_outb = [None, None]
                      for h in range(2):
                          if ccs1:
                              pa16 = papool.tile([16, O], F32, tag=f"pa{h}",
                                                 name="pa16")
                          ex = stage.tile([128, G, D], BF16, tag="ex", name="ex")
                          nc.scalar.activation(ex[:], L[h][:], Exp)
                          sm = small.tile([128, G], F32, tag="sm", name="sm")
                          nc.vector.tensor_reduce(sm[:], ex[:], axis=AX, op=add)
                          rc = small.tile([128, G], F32, tag="rc", name="rc")
                          nc.vector.reciprocal(rc[:], sm[:])
                          R = rpool.tile([128, G, D], BF16, tag="R", name="R")
                          nc.vector.tensor_tensor(
                              R[:], ex[:],
                              rc[:, :, None].to_broadcast([128, G, D]), mult)
                          if wvG0 < G:
                              wvp = wvppool.tile([128, G - wvG0, A, D], BF16,
                                                 tag="wvp", name="wvp")
                              nc.gpsimd.tensor_tensor(
                                  wvp[:], V[h][:, wvG0:],
                                  R[:, wvG0:, None, :]
                                  .to_broadcast([128, G - wvG0, A, D]), mult)
                          for g in range(G):
                              if g >= wvG0:
                                  rhs = wvp[:, g - wvG0]
                              else:
                                  if g % WVC == 0:
                                      wv = wvpool.tile([128, WVC, A, D], BF16,
                                                       tag="wv", name="wv")
                                      nc.vector.tensor_tensor(
                                          wv[:], V[h][:, bass.ts(g // WVC, WVC)],
                                          R[:, bass.ts(g // WVC, WVC), None, :]
                                          .to_broadcast([128, WVC, A, D]), mult)
                                  rhs = wv[:, g % WVC]
                              if ccs1:
                                  nc.tensor.matmul(pa16[:], lhsT=sh_sb[:],
                                                   rhs=rhs[:],
                                                   start=(g == 0),
                                                   stop=(g == G - 1))
                              else:
                                  nc.tensor.matmul(pa32[:],
                                                   lhsT=shx_sb[:, h, :],
                                                   rhs=rhs[:],
                                                   start=(h == 0 and g == 0),
                                                   stop=(h == 1 and g == G - 1))
                          if ccs1:
                              pre_h = small.tile([16, O], cc_dt,
                                                 tag=f"preh{h}", name="pre_h")
                              nc.scalar.copy(pre_h[:], pa16[:])
                              inb = dram.tile([16, O], cc_dt, tag=f"arin{h}",
                                              name="arin")
                              outbh = dram.tile([16, O], cc_dt,
                                                tag=f"arout{h}", name="arout",
                                                addr_space="Shared")
                              nc.sync.dma_start(inb[:], pre_h[:])
                              if "nocc" in opts:
                                  nc.sync.dma_start(outbh[:], inb[:])
                              else:
                                  nc.gpsimd.collective_compute(
                                      "AllReduce", add,
                                      replica_groups=[list(range(N_CORES))],
                                      ins=[inb[:].opt()], outs=[outbh[:].opt()])
                              nxt_outb[h] = outbh[:]
                      if nxt_last:
                          pref32 = pre32p.tile([B, O], F32, tag="pref32",
                                               name="pref32")
                          nc.scalar.copy(pref32[:], pa32[:])
                          nc.sync.dma_start(outp[:], pref32[:])
                      elif ccs1:
                          outb_cur = nxt_outb
                      else:
                          outb32_n = cc_reduce32(pa32)
                          outb_cur = [outb32_n[bass.ts(h, 16)]
                                      for h in range(2)]

              # ---- routing iterations ----
              split = "cc_split" in opts
              phased = "phased" in opts and not hsplit
              n_rt = 0 if ("votes_only" in opts or hsplit) else num_routing
              if phased:
                  # two-phase emission: phase A kicks off softmax/wv/matmuls and
                  # the collectives for BOTH halves; phase B consumes them. This
                  # keeps the other half's DVE work *ahead* of each post-
                  # collective stall in the engine instruction streams.
                  WVC = 4
                  for it in range(n_rt):
                      is_last = it == num_routing - 1
                      cc_dt = BF16 if "cc_bf16" in opts else F32
                      outbs = [None, None]
                      for h in range(2):
                          if dense0 and it == 0:
                              pa = paD[h]
                          else:
                              pa = papool.tile([16, O], F32, tag=f"pa{h}", name="pa")
                          if it > 0:
                              ex = stage.tile([128, G, D], BF16, tag="ex", name="ex")
                              nc.scalar.activation(ex[:], L[h][:], Exp)
                              sm = small.tile([128, G], F32, tag="sm", name="sm")
                              nc.vector.tensor_reduce(sm[:], ex[:], axis=AX, op=add)
                              rc = small.tile([128, G], F32, tag="rc", name="rc")
                              nc.vector.reciprocal(rc[:], sm[:])
                              R = rpool.tile([128, G, D], BF16, tag="R", name="R")
                              nc.vector.tensor_tensor(
                                  R[:], ex[:],
                                  rc[:, :, None].to_broadcast([128, G, D]), mult)
                          wvG0 = G
                          for o in opts:
                              if o.startswith("pswv"):
                                  wvG0 = int(o[4:])
                          if it > 0 and wvG0 < G:
                              # pool-engine slice of the route-weighting
                              wvp = wvppool.tile([128, G - wvG0, A, D], BF16,
                                                 tag="wvp", name="wvp")
                              nc.gpsimd.tensor_tensor(
                                  wvp[:], V[h][:, wvG0:],
                                  R[:, wvG0:, None, :]
                                  .to_broadcast([128, G - wvG0, A, D]), mult)
                          for g in range(G) if not (dense0 and it == 0) else []:
                              if it == 0:
                                  rhs = V[h][:, g]
                              elif g >= wvG0:
                                  rhs = wvp[:, g - wvG0]
                              else:
                                  if g % WVC == 0:
                                      wv = wvpool.tile([128, WVC, A, D], BF16,
                                                       tag="wv", name="wv")
                                      nc.vector.tensor_tensor(
                                          wv[:], V[h][:, bass.ts(g // WVC, WVC)],
                                          R[:, bass.ts(g // WVC, WVC), None, :]
                                          .to_broadcast([128, WVC, A, D]), mult)
                                  rhs = wv[:, g % WVC]
                              lhsT = sh32_sb[:] if it == 0 else sh_sb[:]
                              nc.tensor.matmul(pa[:], lhsT=lhsT, rhs=rhs[:],
                                               start=(g == 0), stop=(g == G - 1))
                          if is_last:
                              pref_h = small.tile([16, O], F32, tag=f"prefh{h}",
                                                  name="pref_h")
                              nc.scalar.copy(pref_h[:], pa[:])
                              nc.sync.dma_start(outp[bass.ts(h, 16), :], pref_h[:])
                              continue
                          if dense0 and it == 0 and d32:
                              # PSUM reads must start at partition 0: copy the
                              # whole [32, O] once, DMA per-half slices
                              if h == 0:
                                  pre32 = pre32p.tile([B, O], cc_dt, tag="pre32",
                                                      name="pre32")
                                  nc.scalar.copy(pre32[:], paD32[:])
                              pre_src = pre32[bass.ts(h, 16)]
                          else:
                              pre_h = small.tile([16, O], cc_dt, tag=f"preh{h}",
                                                 name="pre_h")
                              nc.scalar.copy(pre_h[:], pa[:])
                              pre_src = pre_h[:]
                          inb = dram.tile([16, O], cc_dt, tag=f"arin{h}", name="arin")
                          outb = dram.tile([16, O], cc_dt, tag=f"arout{h}",
                                           name="arout", addr_space="Shared")
                          nc.sync.dma_start(inb[:], pre_src)
                          if "nocc" in opts:
                              nc.sync.dma_start(outb[:], inb[:])
                          else:
                              nc.gpsimd.collective_compute(
                                  "AllReduce", add,
                                  replica_groups=[list(range(N_CORES))],
                                  ins=[inb[:].opt()], outs=[outb[:].opt()])
                          outbs[h] = outb
                      if is_last:
                          continue
                      if "ilv" in opts:
                          # step-interleave both halves' squash chains so each
                          # ACT<->DVE hop is overlapped by the other half's op
                          t2s, sqs, n2s, nrms = [], [], [], []
                          dens, rc2s, facs, actbs = [], [], [], []
                          for h in range(2):
                              ob_sb = small.tile([16, O], cc_dt, tag=f"ob{h}",
                                                 name="ob_sb")
                              nc.sync.dma_start(ob_sb[:], outbs[h][:])
                              prep_ps = papool.tile([128, O], F32, tag=f"prps{h}",
                                                    name="prep_ps")
                              nc.tensor.matmul(prep_ps[:], lhsT=repl_sb[:],
                                               rhs=ob_sb[:], start=True, stop=False)
                              nc.tensor.matmul(prep_ps[:], lhsT=ones1_sb[:],
                                               rhs=bias2_sb[:], start=False,
                                               stop=True)
                              t2s.append(prep_ps[:].rearrange("p (a d) -> p a d",
                                                              a=A))
                          for h in range(2):
                              sq = stage.tile([128, A, D], F32, tag="sq", name="sq")
                              nc.scalar.activation(sq[:], t2s[h], Square)
                              sqs.append(sq)
                          for h in range(2):
                              n2 = small.tile([128, D], F32, tag="n2", name="n2")
                              nc.vector.tensor_reduce(
                                  n2[:], sqs[h][:].rearrange("p a d -> p d a"),
                                  axis=AX, op=add)
                              n2s.append(n2)
                          for h in range(2):
                              nrm = small.tile([128, D], F32, tag="nrm", name="nrm")
                              nc.scalar.activation(nrm[:], n2s[h][:], Sqrt)
                              nrms.append(nrm)
                          for h in range(2):
                              den = small.tile([128, D], F32, tag="den", name="den")
                              nc.scalar.add(den[:], n2s[h][:], 1.0)
                              dens.append(den)
                          for h in range(2):
                              rc2 = small.tile([128, D], F32, tag="rc2", name="rc2")
                              nc.vector.reciprocal(rc2[:], dens[h][:])
                              rc2s.append(rc2)
                          for h in range(2):
                              fac = small.tile([128, D], F32, tag="fac", name="fac")
                              nc.vector.tensor_tensor(fac[:], nrms[h][:],
                                                      rc2s[h][:], mult)
                              facs.append(fac)
                          for h in range(2):
                              actb = stage.tile([128, A, D], BF16, tag="actb",
                                                name="actb")
                              nc.vector.tensor_tensor(
                                  actb[:], t2s[h],
                                  facs[h][:, None, :].to_broadcast([128, A, D]),
                                  mult)
                              actbs.append(actb)
                          for h in range(2):
                              u = upool.tile([128, G, A, D], BF16, tag=f"u{h}",
                                             name="u")
                              nc.vector.tensor_tensor(
                                  u[:], V[h][:],
                                  actbs[h][:, None, :, :]
                                  .to_broadcast([128, G, A, D]), mult)
                              half = A // 2
                              while half >= 1:
                                  nc.vector.tensor_tensor(
                                      u[:, :, 0:half], u[:, :, 0:half],
                                      u[:, :, half:2 * half], add)
                                  half //= 2
                              nc.vector.tensor_tensor(L[h][:], L[h][:],
                                                      u[:, :, 0, :], add)
                          continue
                      for h in range(2):
                          ob_sb = small.tile([16, O], cc_dt, tag=f"ob{h}",
                                             name="ob_sb")
                          nc.sync.dma_start(ob_sb[:], outbs[h][:])
                          prep_ps = papool.tile([128, O], F32, tag=f"prps{h}",
                                                name="prep_ps")
                          if "tail1" in opts:
                              nc.tensor.matmul(prep_ps[:], lhsT=repl_sb[:],
                                               rhs=ob_sb[:], start=True, stop=False)
                              nc.tensor.matmul(prep_ps[:], lhsT=ones1_sb[:],
                                               rhs=bias2_sb[:], start=False,
                                               stop=True)
                              t2 = prep_ps[:].rearrange("p (a d) -> p a d", a=A)
                          else:
                              nc.tensor.matmul(prep_ps[:], lhsT=repl_sb[:],
                                               rhs=ob_sb[:], start=True, stop=True)
                              prep = prep_ps[:].rearrange("p (a d) -> p a d", a=A)
                              t2f = stage.tile([128, A, D], F32, tag="t2", name="t2")
                              nc.vector.tensor_tensor(t2f[:], prep, bias_sb[:], add)
                              t2 = t2f[:]
                          sq = stage.tile([128, A, D], F32, tag="sq", name="sq")
                          nc.scalar.activation(sq[:], t2, Square)
                          n2 = small.tile([128, D], F32, tag="n2", name="n2")
                          nc.vector.tensor_reduce(
                              n2[:], sq[:].rearrange("p a d -> p d a"),
                              axis=AX, op=add)
                          nrm = small.tile([128, D], F32, tag="nrm", name="nrm")
                          nc.scalar.activation(nrm[:], n2[:], Sqrt)
                          den = small.tile([128, D], F32, tag="den", name="den")
                          nc.vector.tensor_scalar_add(den[:], n2[:], 1.0)
                          rc2 = small.tile([128, D], F32, tag="rc2", name="rc2")
                          nc.vector.reciprocal(rc2[:], den[:])
                          fac = small.tile([128, D], F32, tag="fac", name="fac")
                          nc.vector.tensor_tensor(fac[:], nrm[:], rc2[:], mult)
                          actb = stage.tile([128, A, D], BF16, tag="actb",
                                            name="actb")
                          nc.vector.tensor_tensor(
                              actb[:], t2,
                              fac[:, None, :].to_broadcast([128, A, D]), mult)
                          uG0 = G
                          for o in opts:
                              if o.startswith("psu"):
                                  uG0 = int(o[3:])
                          if uG0 < G:
                              # pool-engine slice of the agreement update
                              up = uppool.tile([128, G - uG0, A, D], BF16,
                                               tag="up", name="up")
                              nc.gpsimd.tensor_tensor(
                                  up[:], V[h][:, uG0:],
                                  actb[:, None, :, :]
                                  .to_broadcast([128, G - uG0, A, D]), mult)
                          u = upool.tile([128, uG0, A, D], BF16, tag="u",
                                         name="u")
                          nc.vector.tensor_tensor(
                              u[:], V[h][:, :uG0],
                              actb[:, None, :, :].to_broadcast([128, uG0, A, D]),
                              mult)
                          half = A // 2
                          while half >= 1:
                              if uG0 < G:
                                  nc.gpsimd.tensor_tensor(
                                      up[:, :, 0:half], up[:, :, 0:half],
                                      up[:, :, half:2 * half], add)
                              nc.vector.tensor_tensor(
                                  u[:, :, 0:half], u[:, :, 0:half],
                                  u[:, :, half:2 * half], add)
                              half //= 2
                          nc.vector.tensor_tensor(L[h][:, :uG0], L[h][:, :uG0],
                                                  u[:, :, 0, :], add)
                          if uG0 < G:
                              nc.gpsimd.tensor_tensor(L[h][:, uG0:],
                                                      L[h][:, uG0:],
                                                      up[:, :, 0, :], add)
              for it in range(0 if phased else n_rt):
                  is_last = it == num_routing - 1
                  cc_dt = BF16 if "cc_bf16" in opts else F32
                  if not split:
                      pa = papool.tile([B, O], F32, tag="pa", name="pa")
                  n_mm = 0
                  total_mm = 2 * G
                  for h in range(2):
                      if split:
                          pa = papool.tile([16, O], F32, tag=f"pa{h}", name="pa")
                          n_mm = 0
                          total_mm = G
                      if it > 0:
                          # softmax over d on L[h] -> R (bf16)
                          ex = stage.tile([128, G, D], F32, tag="ex", name="ex")
                          if "noms" in opts:
                              # logits are bounded (|L| < ~8) so exp is safe
                              # in fp32 without the max-subtraction
                              nc.scalar.activation(ex[:], L[h][:], Exp)
                          else:
                              mx = small.tile([128, G], F32, tag="mx", name="mx")
                              nc.vector.tensor_reduce(mx[:], L[h][:], axis=AX, op=amax)
                              nc.vector.tensor_tensor(
                                  ex[:], L[h][:],
                                  mx[:, :, None].to_broadcast([128, G, D]), sub)
                              nc.scalar.activation(ex[:], ex[:], Exp)
                          sm = small.tile([128, G], F32, tag="sm", name="sm")
                          nc.vector.tensor_reduce(sm[:], ex[:], axis=AX, op=add)
                          rc = small.tile([128, G], F32, tag="rc", name="rc")
                          nc.vector.reciprocal(rc[:], sm[:])
                          R = rpool.tile([128, G, D], BF16, tag="R", name="R")
                          nc.vector.tensor_tensor(
                              R[:], ex[:],
                              rc[:, :, None].to_broadcast([128, G, D]), mult)
                          if dbg and it == 1:
                              nc.sync.dma_start(
                                  dbg_R[h].rearrange("p (g d) -> p g d", g=G), R[:])
                      WVC = 4  # g-groups per wv chunk
                      for g in range(G):
                          if it == 0:
                              rhs = V[h][:, g]
                          else:
                              if g % WVC == 0:
                                  wv = wvpool.tile([128, WVC, A, D], BF16,
                                                  tag="wv", name="wv")
                                  in1 = (V[h][:, bass.ts(g // WVC, WVC)]
                                         if "bcast_probe" in opts else
                                         R[:, bass.ts(g // WVC, WVC), None, :]
                                         .to_broadcast([128, WVC, A, D]))
                                  nc.vector.tensor_tensor(
                                      wv[:], V[h][:, bass.ts(g // WVC, WVC)],
                                      in1, mult)
                              rhs = wv[:, g % WVC]
                          if split:
                              lhsT = sh32_sb[:] if it == 0 else sh_sb[:]
                          else:
                              lhsT = (s32_sb if it == 0 else s_sb)[:, h, :]
                          nc.tensor.matmul(pa[:], lhsT=lhsT, rhs=rhs[:],
                                           start=(n_mm == 0), stop=(n_mm == total_mm - 1))
                          n_mm += 1
                      if not split:
                          continue
                      # ---- per-half collective + squash + update ----
                      nh = 16
                      pre_h = small.tile([nh, O], cc_dt, tag=f"preh{h}", name="pre_h")
                      if is_last:
                          pref_h = small.tile([nh, O], F32, tag=f"prefh{h}", name="pref_h")
                          nc.scalar.copy(pref_h[:], pa[:])
                          nc.sync.dma_start(outp[bass.ts(h, nh), :], pref_h[:])
                          continue
                      nc.scalar.copy(pre_h[:], pa[:])
                      inb = dram.tile([nh, O], cc_dt, tag=f"arin{h}", name="arin")
                      outb = dram.tile([nh, O], cc_dt, tag=f"arout{h}", name="arout",
                                       addr_space="Shared")
                      nc.sync.dma_start(inb[:], pre_h[:])
                      if "nocc" in opts:
                          nc.sync.dma_start(outb[:], inb[:])
                      else:
                          nc.gpsimd.collective_compute(
                              "AllReduce", add,
                              replica_groups=[list(range(N_CORES))],
                              ins=[inb[:].opt()], outs=[outb[:].opt()])
                      upd = nc.vector
                      if "pe_repl" in opts:
                          ob_sb = small.tile([16, O], cc_dt, tag=f"ob{h}", name="ob_sb")
                          nc.sync.dma_start(ob_sb[:], outb[:])
                          prep_ps = papool.tile([128, O], F32, tag=f"prps{h}",
                                                name="prep_ps")
                          nc.tensor.matmul(prep_ps[:], lhsT=repl_sb[:], rhs=ob_sb[:],
                                           start=True, stop=True)
                          prep = prep_ps[:].rearrange("p (a d) -> p a d", a=A)
                      else:
                          prep_t = stage.tile([128, A, D], cc_dt, tag="prep",
                                              name="prep")
                          for j in range(8):
                              nc.sync.dma_start(
                                  prep_t[bass.ts(j, 16)].rearrange("b a d -> b (a d)"),
                                  outb[:])
                          prep = prep_t[:]
                      t2 = stage.tile([128, A, D], F32, tag="t2", name="t2")
                      nc.vector.tensor_tensor(t2[:], prep, bias_sb[:], add)
                      sq = stage.tile([128, A, D], F32, tag="sq", name="sq")
                      nc.scalar.activation(sq[:], t2[:], Square)
                      n2 = small.tile([128, D], F32, tag="n2", name="n2")
                      nc.vector.tensor_reduce(
                          n2[:], sq[:].rearrange("p a d -> p d a"), axis=AX, op=add)
                      nrm = small.tile([128, D], F32, tag="nrm", name="nrm")
                      nc.scalar.activation(nrm[:], n2[:], Sqrt)
                      den = small.tile([128, D], F32, tag="den", name="den")
                      nc.vector.tensor_scalar_add(den[:], n2[:], 1.0)
                      rc2 = small.tile([128, D], F32, tag="rc2", name="rc2")
                      nc.vector.reciprocal(rc2[:], den[:])
                      fac = small.tile([128, D], F32, tag="fac", name="fac")
                      nc.vector.tensor_tensor(fac[:], nrm[:], rc2[:], mult)
                      actb = stage.tile([128, A, D], BF16, tag="actb", name="actb")
                      nc.vector.tensor_tensor(
                          actb[:], t2[:],
                          fac[:, None, :].to_broadcast([128, A, D]), mult)
                      u = upool.tile([128, G, A, D], BF16, tag=f"u{h}", name="u")
                      uin1 = (V[h][:] if "bcast_probe" in opts else
                              actb[:, None, :, :].to_broadcast([128, G, A, D]))
                      upd.tensor_tensor(u[:], V[h][:], uin1, mult)
                      half = A // 2
                      while half >= 1:
                          upd.tensor_tensor(
                              u[:, :, 0:half], u[:, :, 0:half],
                              u[:, :, half:2 * half], add)
                          half //= 2
                      upd.tensor_tensor(L[h][:], L[h][:], u[:, :, 0, :], add)

                  if split:
                      continue
                  pre_sb = small.tile([B, O], cc_dt, tag="pre", name="pre_sb")
                  if is_last:
                      pre_f32 = small.tile([B, O], F32, tag="pref", name="pre_f32")
                      nc.scalar.copy(pre_f32[:], pa[:])
                      nc.sync.dma_start(outp[:], pre_f32[:])
                      continue
                  nc.scalar.copy(pre_sb[:], pa[:])

                  inb = dram.tile([B, O], cc_dt, tag="arin", name="arin")
                  outb = dram.tile([B, O], cc_dt, tag="arout", name="arout",
                                   addr_space="Shared")
                  nc.sync.dma_start(inb[:], pre_sb[:])
                  if "nocc" in opts:
                      nc.sync.dma_start(outb[:], inb[:])
                  else:
                      nc.gpsimd.collective_compute(
                          "AllReduce", add,
                          replica_groups=[list(range(N_CORES))],
                          ins=[inb[:].opt()], outs=[outb[:].opt()])
                  if dbg and it == 0:
                      nc.sync.dma_start(dbg_ar[:], outb[:])

                  for h in range(2):
                      upd = nc.gpsimd if ("gps_h1" in opts and h == 1) else nc.vector
                      # replicate the 16 b-rows of this half across 8 j-groups
                      prep = stage.tile([128, A, D], cc_dt, tag="prep", name="prep")
                      for j in range(8):
                          nc.sync.dma_start(
                              prep[bass.ts(j, 16)].rearrange("b a d -> b (a d)"),
                              outb[bass.ts(h, 16), :])
                      if dbg and it == 0:
                          nc.sync.dma_start(
                              dbg_prep[h].rearrange("p (a d) -> p a d", a=A), prep[:])
                      # t = preact + bias
                      t2 = stage.tile([128, A, D], F32, tag="t2", name="t2")
                      nc.vector.tensor_tensor(t2[:], prep[:], bias_sb[:], add)
                      # squash factor f = n / (1 + n^2), n = ||t|| over a
                      sq = stage.tile([128, A, D], F32, tag="sq", name="sq")
                      nc.scalar.activation(sq[:], t2[:], Square)
                      n2 = small.tile([128, D], F32, tag="n2", name="n2")
                      nc.vector.tensor_reduce(
                          n2[:], sq[:].rearrange("p a d -> p d a"), axis=AX, op=add)
                      nrm = small.tile([128, D], F32, tag="nrm", name="nrm")
                      nc.scalar.activation(nrm[:], n2[:], Sqrt)
                      den = small.tile([128, D], F32, tag="den", name="den")
                      nc.vector.tensor_scalar_add(den[:], n2[:], 1.0)
                      rc2 = small.tile([128, D], F32, tag="rc2", name="rc2")
                      nc.vector.reciprocal(rc2[:], den[:])
                      fac = small.tile([128, D], F32, tag="fac", name="fac")
                      nc.vector.tensor_tensor(fac[:], nrm[:], rc2[:], mult)
                      actb = stage.tile([128, A, D], BF16, tag="actb", name="actb")
                      nc.vector.tensor_tensor(
                          actb[:], t2[:],
                          fac[:, None, :].to_broadcast([128, A, D]), mult)
                      # update: L[h] += sum_a V * actb
                      u = upool.tile([128, G, A, D], BF16, tag=f"u{h}", name="u")
                      upd.tensor_tensor(
                          u[:], V[h][:],
                          actb[:, None, :, :].to_broadcast([128, G, A, D]), mult)
                      if "gps_pool1" in opts and h == 1:
                          pooled = stage.tile([128, G, D], F32, tag="pooled",
                                              name="pooled")
                          nc.gpsimd.pool(pooled[:],
                                         u[:].rearrange("p g a d -> p g d a"),
                                         func=mybir.PoolFunctionType.avg)
                          nc.vector.scalar_tensor_tensor(
                              L[h][:], pooled[:], float(A), L[h][:], mult, add)
                      else:
                          half = A // 2
                          while half >= 1:
                              upd.tensor_tensor(
                                  u[:, :, 0:half], u[:, :, 0:half],
                                  u[:, :, half:2 * half], add)
                              half //= 2
                          upd.tensor_tensor(L[h][:], L[h][:], u[:, :, 0, :], add)
                      if dbg and it == 0:
                          nc.sync.dma_start(
                              dbg_actb[h].rearrange("p (a d) -> p a d", a=A), actb[:])
                          nc.sync.dma_start(
                              dbg_L[h].rearrange("p (g d) -> p g d", g=G), L[h][:])

    nc.compile()
    return nc


KERNEL_OPTS = frozenset(["noms", "cc_bf16", "w_bf16", "cc_split", "pe_repl",
                         "phased", "tail1", "dense0", "d32", "xdc", "wb2",
                         "wb4", "xd8"])


@functools.lru_cache(maxsize=4)
def _get_compiled(num_routing: int):
    return _build(num_routing, opts=KERNEL_OPTS)


def _host_inputs(x, weights, opts: frozenset = frozenset()):
    """Build the per-core input maps (everything except tiny constants)."""
    dt = _nbf16 if "w_bf16" in opts else np.float32
    x_np = np.ascontiguousarray(x.reshape(B, I, C), dtype=np.float32)
    # o' = a*D + d ordering
    w2 = np.ascontiguousarray(
        weights.reshape(I, C, D, A).transpose(0, 1, 3, 2), dtype=np.float32)
    if dt is not np.float32:
        x_np = x_np.astype(dt)
        w2 = w2.astype(dt)

    in_maps = []
    for r in range(N_CORES):
        sl = slice(r * I_LOC, (r + 1) * I_LOC)
        w_r = w2[sl].reshape(I_LOC * C, O)
        # xd[g, h, j*16+c, j*16+bh] = x[h*16+bh, r*I_LOC + g*8 + j, c]
        arr = x_np[:, sl, :].reshape(2, 16, G, 8, C)  # (h, bh, g, j, c)
        xd = np.zeros((G, 2, 128, 128), dt)
        for j in range(8):
            xd[:, :, j * 16:(j + 1) * 16, j * 16:(j + 1) * 16] = \
                arr[:, :, :, j, :].transpose(2, 0, 3, 1)  # (g, h, c, bh)
        # xt[g, (j, c), h, bh] = x[h*16+bh, r*I_LOC + g*8 + j, c] / D
        xt = np.ascontiguousarray(
            (arr.astype(np.float32) / D).transpose(2, 3, 4, 0, 1)
            .reshape(G, 128, 2, 16)).astype(_nbf16)
        m = {"xt": xt}
        if "wb2" in opts:
            m["wh"] = np.ascontiguousarray(
                w_r.reshape(G, 128, O).transpose(1, 0, 2))
        else:
            m["w"] = np.ascontiguousarray(w_r)
        if "xdc" in opts:
            xb = 8 if "xd8" in opts else 4
            m["xdc"] = np.ascontiguousarray(
                xd.reshape(G // xb, xb, 2, 128, 128).transpose(0, 3, 1, 2, 4))
        else:
            m["xd"] = xd
        if "d32" in opts:
            m["xt32"] = np.ascontiguousarray(
                (arr.astype(np.float32) / D).transpose(3, 4, 2, 0, 1)
                .reshape(128, G, 32)).astype(_nbf16)
        in_maps.append(m)
    return in_maps


def _host_constants(bias):
    # S[h, j*16+bh, b] = (b == h*16+bh)
    s = np.zeros((128, 2, B), np.float32)
    sh = np.zeros((128, 16), np.float32)
    for h in range(2):
        for j in range(8):
            for bh in range(16):
                s[j * 16 + bh, h, h * 16 + bh] = 1.0
    for j in range(8):
        for bh in range(16):
            sh[j * 16 + bh, bh] = 1.0
    repl = np.zeros((16, 128), np.float32)
    for j in range(8):
        for bh in range(16):
            repl[bh, j * 16 + bh] = 1.0
    shx = np.zeros((128, 2, 32), np.float32)
    for h in range(2):
        for j in range(8):
            for bh in range(16):
                shx[j * 16 + bh, h, h * 16 + bh] = 1.0
    bias2 = np.ascontiguousarray(bias.reshape(D, A).T, dtype=np.float32).reshape(O)
    bias_bc = np.tile(bias2[None, :], (128, 1)).astype(np.float32)
    return {"s": s.astype(_nbf16), "s32": (s / D).astype(_nbf16),
            "shx": shx.astype(_nbf16),
            "sh": sh.astype(_nbf16), "sh32": (sh / D).astype(_nbf16),
            "repl": repl.astype(_nbf16), "biasb": bias_bc,
            "bias2": bias2[None, :].astype(_nbf16),
            "ones1": np.ones((1, 128), _nbf16)}


def _squash_host(t):
    # t: [B, D, A] float64; squash over a
    n2 = (t ** 2).sum(axis=2, keepdims=True)
    n = np.sqrt(n2)
    return t * (n / (1.0 + n2))


def kernel(x, weights, bias, num_routing):
    n = int(num_routing)
    x = np.asarray(x, dtype=np.float32)
    weights = np.asarray(weights, dtype=np.float32)
    bias_np = np.asarray(bias, dtype=np.float32)

    nc = _get_compiled(n)
    in_maps = _host_inputs(x, weights, opts=KERNEL_OPTS)
    consts = _host_constants(bias_np)
    for m in in_maps:
        m.update(consts)

    # the axon tunnel occasionally returns a transient
    # NRT_EXEC_UNIT_UNRECOVERABLE; one retry has recovered every observed case
    import time as _time
    try:
        res = bass_utils.run_bass_kernel_spmd(
            nc, in_maps, core_ids=list(range(N_CORES)))
    except Exception:
        _time.sleep(10)
        res = bass_utils.run_bass_kernel_spmd(
            nc, in_maps, core_ids=list(range(N_CORES)))

    partials = np.stack([res.results[r]["outp"] for r in range(N_CORES)], axis=0)
    pre = partials.astype(np.float64).sum(axis=0)            # [B, O] in (a, d)
    pre = pre.reshape(B, A, D).transpose(0, 2, 1)            # [B, D, A]
    pre = pre + bias_np.reshape(D, A)[None].astype(np.float64)
    act = _squash_host(pre).astype(np.float32)
    return act.reshape(B, D, A, 1, 1)


if __name__ == "__main__":
    import sys
    sys.path.insert(0, "/root/problem")
    from reference import setup_inputs, reference

    inputs = {k: np.asarray(v) if not isinstance(v, int) else v
              for k, v in setup_inputs().items()}
    ref = np.asarray(reference(**inputs))
    out = kernel(**inputs)
    d = np.abs(out - ref)
    print("absmax", d.max(), "ref absmax", np.abs(ref).max(),
          "scale-rel", d.max() / np.abs(ref).max(),
          "rel_l2", np.linalg.norm(d) / np.linalg.norm(ref))

